# revision 16
# baseline (speedup 1.0000x reference)
"""GQA attention kernel for 8 Trainium2 NeuronCores.

Problem: B=2, N=2048, D=2048, H=32 heads, G=8 KV groups, head_dim=64, RoPE,
causal mask, fused QKV/output projections.

Sharding: one (batch, group-pair) unit per core — core c handles batch c//4
and KV groups {2*(c%4), 2*(c%4)+1} (8 query heads). Each core computes a
partial output projection (its heads' rows of Wo); the host sums the 4
partials per batch.

Per-core pipeline (all matmuls in bf16, fp32 accumulate):
  x --cast DMA--> xbf (DRAM, bf16) --xbar-transpose DMA--> xT [din, tok]
  QKV projections (lhsT = xT blocks), RoPE in natural layout on DVE,
  PE-transpose q/k to q^T/k^T [d, tok], then attention per token-half
  (1024 tokens) per head, key-block-major:
    scores^T[m] = k^T_m.T @ q^T  (PSUM) -> exp on ACT -> attn^T (bf16 SBUF)
    causal: skip key blocks above the diagonal; triangular mask on diag block
    ctx^T += [v_m | 1].T @ attn^T_m  -> row 64 = softmax denominators
  normalize: reciprocal_approx_fast on the denominator row, gpsimd
  partition_broadcast across 64 partitions, one DVE multiply into ctx^T
  (psx double-buffered so the tail overlaps the next head's matmuls),
  out = ctx^T.T @ Wo in bf16 partials summed on host.
"""

import numpy as np

import concourse.bass as bass
import concourse.bacc as bacc
import concourse.mybir as mybir
import concourse.tile as tile
from concourse.bass_utils import run_bass_kernel_spmd
from concourse.masks import make_identity, make_upper_triangular

F32 = mybir.dt.float32
BF16 = mybir.dt.bfloat16

N = 2048          # sequence length
D = 2048          # model dim
HD = 64           # head dim
QF = 512          # q features per core (8 heads)
KF = 128          # k/v features per core (2 groups)
NT = N // 128     # token blocks
KC = D // 128     # contraction chunks
NH = N // 2       # tokens per attention half
SCALE = 1.0 / 8.0  # 1/sqrt(HD)


def _build_program():
    nc = bacc.Bacc("TRN2", debug=False, target_bir_lowering=False)

    x_d = nc.dram_tensor("x", [N, D], F32, kind="ExternalInput")
    cos_d = nc.dram_tensor("cos", [N, HD], F32, kind="ExternalInput")
    sin_d = nc.dram_tensor("sin", [N, HD], F32, kind="ExternalInput")
    wq_d = nc.dram_tensor("wq", [D, QF], F32, kind="ExternalInput")
    wk_d = nc.dram_tensor("wk", [D, KF], F32, kind="ExternalInput")
    wv_d = nc.dram_tensor("wv", [D, KF], F32, kind="ExternalInput")
    wo_d = nc.dram_tensor("wo", [QF, D], F32, kind="ExternalInput")
    out_d = nc.dram_tensor("out", [N, D], BF16, kind="ExternalOutput")

    with tile.TileContext(nc) as tc:
        with tc.tile_pool(name="persist", bufs=1) as pp:
            # persistent SBUF: q^T/k^T, [v|1], ctx^T, wo, constants
            qT = [pp.tile([128, N], BF16, name=f"qT{t}") for t in range(4)]
            kT = pp.tile([128, N], BF16, name="kT")
            vo = [pp.tile([128, NT, HD + 1], BF16, name=f"vo{g}") for g in range(2)]
            ctxT = [pp.tile([128, N], BF16, name=f"ctxT{k}") for k in range(4)]
            wo_sb = pp.tile([128, 4, N], BF16, name="wo_sb")
            ident = pp.tile([128, 128], BF16, name="ident")
            maskn = pp.tile([128, 128], BF16, name="maskn")
            junk = pp.tile([128, 512], BF16, name="junk")
            jex = pp.tile([128, 1], F32, name="jex")

            make_identity(nc, ident)
            # strict upper triangle of -1e9: maskn.T @ I added to the
            # diagonal score block zeroes masked positions through exp
            make_upper_triangular(nc, maskn, val=-1e9, diag=False)
            nc.vector.memset(junk[:], 1.0)
            # preload the exp table set during the DMA preamble
            nc.scalar.activation(jex[:], junk[:, 0:1],
                                 mybir.ActivationFunctionType.Exp)
            for g in range(2):
                nc.vector.memset(vo[g][:, :, HD:HD + 1], 1.0)

            nc.gpsimd.dma_start(
                wo_sb[:], wo_d[:].rearrange("(ko ki) n -> ki ko n", ki=128))

            # ---------------- phase A: x^T + projections + rope ----------
            with tc.tile_pool(name="phaseA", bufs=1) as pa, \
                 tc.tile_pool(name="dram", bufs=1, space="DRAM") as dp, \
                 tc.tile_pool(name="ps_q", bufs=2, space="PSUM") as ps_q, \
                 tc.tile_pool(name="ps_kv", bufs=2, space="PSUM") as ps_kv, \
                 tc.tile_pool(name="ps_tr", bufs=2, space="PSUM") as ps_tr, \
                 tc.tile_pool(name="ropetmp", bufs=6) as rtp:

                xbf = dp.tile([N, D], BF16, name="xbf")
                xT = [pa.tile([128, N], BF16, name=f"xT{kc}")
                      for kc in range(KC)]
                wq_sb = pa.tile([128, KC, QF], BF16, name="wq_sb")
                wkv_sb = pa.tile([128, KC, 2 * KF], BF16, name="wkv_sb")
                cos_sb = pa.tile([128, NT, HD], F32, name="cos_sb")
                sin_sb = pa.tile([128, NT, HD], F32, name="sin_sb")
                q_rope = pa.tile([128, NT, QF], BF16, name="q_rope")
                k_rope = pa.tile([128, NT, KF], BF16, name="k_rope")

                nc.gpsimd.dma_start(
                    wq_sb[:], wq_d[:].rearrange("(ko ki) n -> ki ko n", ki=128))
                nc.gpsimd.dma_start(
                    wkv_sb[:, :, 0:KF],
                    wk_d[:].rearrange("(ko ki) n -> ki ko n", ki=128))
                nc.gpsimd.dma_start(
                    wkv_sb[:, :, KF:2 * KF],
                    wv_d[:].rearrange("(ko ki) n -> ki ko n", ki=128))
                nc.sync.dma_start(
                    cos_sb[:], cos_d[:].rearrange("(t p) d -> p t d", p=128))
                nc.sync.dma_start(
                    sin_sb[:], sin_d[:].rearrange("(t p) d -> p t d", p=128))

                # cast x to bf16 in DRAM in 4 column groups so the
                # xbar-transposes (which need all rows of their column
                # block) can start after 1/4 of the cast
                for cg in range(4):
                    nc.gpsimd.dma_start(xbf[:, cg * 512:(cg + 1) * 512],
                                        x_d[:, cg * 512:(cg + 1) * 512])
                for kc in range(KC):
                    nc.sync.dma_start_transpose(
                        xT[kc][:], xbf[:, kc * 128:(kc + 1) * 128])

                def rope(ps, cos_b, sin_b, out_v, ab_shape):
                    """ps 4D view [128, *ab, 2, 32]; cos_b/sin_b broadcast
                    [128, *ab, 32]; out_v same 4D view layout as ps."""
                    q1 = ps[..., 0, :]
                    q2 = ps[..., 1, :]
                    c1 = cos_b[0]
                    c2 = cos_b[1]
                    s1 = sin_b[0]
                    s2 = sin_b[1]
                    ta = rtp.tile([128] + ab_shape + [32], F32, name="rt", tag="rt")
                    tb = rtp.tile([128] + ab_shape + [32], F32, name="rt", tag="rt")
                    nc.vector.tensor_mul(ta[:], q1, c1)
                    nc.vector.tensor_mul(tb[:], q2, s1)
                    nc.vector.tensor_sub(out_v[..., 0, :], ta[:], tb[:])
                    tc_ = rtp.tile([128] + ab_shape + [32], F32, name="rt", tag="rt")
                    td = rtp.tile([128] + ab_shape + [32], F32, name="rt", tag="rt")
                    nc.vector.tensor_mul(tc_[:], q2, c2)
                    nc.vector.tensor_mul(td[:], q1, s2)
                    nc.vector.tensor_add(out_v[..., 1, :], tc_[:], td[:])

                for tb_i in range(NT):
                    psq = ps_q.tile([128, QF], F32, name="psq", tag="psq")
                    pskv = ps_kv.tile([128, 2 * KF], F32, name="pskv", tag="pskv")
                    for kc in range(KC):
                        lhsT = xT[kc][:, tb_i * 128:(tb_i + 1) * 128]
                        st = kc == 0
                        sp = kc == KC - 1
                        nc.tensor.matmul(psq[:], lhsT, wq_sb[:, kc, :],
                                         start=st, stop=sp)
                        nc.tensor.matmul(pskv[:], lhsT, wkv_sb[:, kc, :],
                                         start=st, stop=sp)

                    # --- RoPE Q: psq cols = a*256 + b*64 + h*32 + j
                    #     out cols = b*128 + a*64 + h*32 + j (head pairs
                    #     (t, t+4) adjacent for the transpose step)
                    psq_v = psq[:].rearrange("p (a b h j) -> p a b h j",
                                             a=2, b=4, h=2)
                    out_v = q_rope[:, tb_i, :].rearrange(
                        "p (b a h j) -> p a b h j", b=4, a=2, h=2)
                    cs = cos_sb[:, tb_i, :]
                    sn = sin_sb[:, tb_i, :]

                    def bcq(apv):
                        return apv.unsqueeze(1).unsqueeze(1).broadcast_to(
                            (128, 2, 4, 32))

                    rope(psq_v,
                         (bcq(cs[:, 0:32]), bcq(cs[:, 32:64])),
                         (bcq(sn[:, 0:32]), bcq(sn[:, 32:64])),
                         out_v, [2, 4])

                    # --- RoPE K: cols = g*64 + h*32 + j (no interleave)
                    psk_v = pskv[:, 0:KF].rearrange("p (g h j) -> p g h j",
                                                    g=2, h=2)
                    outk_v = k_rope[:, tb_i, :].rearrange(
                        "p (g h j) -> p g h j", g=2, h=2)

                    def bck(apv):
                        return apv.unsqueeze(1).broadcast_to((128, 2, 32))

                    rope(psk_v,
                         (bck(cs[:, 0:32]), bck(cs[:, 32:64])),
                         (bck(sn[:, 0:32]), bck(sn[:, 32:64])),
                         outk_v, [2])

                    # --- V -> bf16 SBUF with ones column
                    for g in range(2):
                        nc.scalar.copy(vo[g][:, tb_i, 0:HD],
                                       pskv[:, KF + g * 64:KF + (g + 1) * 64])

                    # --- PE transposes: q_rope/k_rope -> qT/kT
                    for t in range(4):
                        ptr = ps_tr.tile([128, 128], BF16, name="ptr", tag="ptr")
                        nc.tensor.transpose(
                            ptr[:], q_rope[:, tb_i, t * 128:(t + 1) * 128],
                            ident[:])
                        nc.vector.tensor_copy(
                            qT[t][:, tb_i * 128:(tb_i + 1) * 128], ptr[:])
                    ptrk = ps_tr.tile([128, 128], BF16, name="ptr", tag="ptr")
                    nc.tensor.transpose(ptrk[:], k_rope[:, tb_i, :], ident[:])
                    nc.scalar.copy(kT[:, tb_i * 128:(tb_i + 1) * 128], ptrk[:])

            # ---------------- phase B: attention ------------------------
            # token-half-major so the normalization tail of head l
            # overlaps the matmuls of head l+1 (psx double-buffered)
            with tc.tile_pool(name="ps_sc", bufs=2, space="PSUM") as ps_sc, \
                 tc.tile_pool(name="ps_cx", bufs=2, space="PSUM") as ps_cx, \
                 tc.tile_pool(name="attnp", bufs=3) as ap_, \
                 tc.tile_pool(name="normp", bufs=2) as np_:

                def attn_head(h, l):
                    t0 = h * NH
                    t1c = (h + 1) * NH
                    m_hi = (h + 1) * (NH // 128)   # key blocks 0..m_hi-1
                    a, b = l // 4, l % 4
                    r0 = 64 * a
                    psx = ps_cx.tile([HD + 1, NH], F32, name="psx",
                                     tag="psx")
                    for m in range(m_hi):
                        g0 = max(m * 128, t0)
                        w = t1c - g0
                        lhs_k = kT[r0:r0 + 64, m * 128:(m + 1) * 128]
                        psc = ps_sc.tile([128, NH], F32, name="psc",
                                         tag="psc")
                        off = 0
                        while off < w:
                            nw = min(512, w - off)
                            diag = (g0 == m * 128 and off == 0)
                            nc.tensor.matmul(
                                psc[:, off:off + nw], lhs_k,
                                qT[b][r0:r0 + 64,
                                      g0 + off:g0 + off + nw],
                                start=True, stop=not diag)
                            if diag:
                                # causal mask on the diagonal block, done
                                # on the PE so the exp->ctx chain never
                                # waits on the vector queue
                                nc.tensor.matmul(
                                    psc[:, 0:128], maskn[:], ident[:],
                                    start=False, stop=True,
                                    skip_group_check=True)
                            off += nw
                        at = ap_.tile([128, NH], BF16, name="at",
                                      tag="at")
                        nc.scalar.activation(
                            at[:, :w], psc[:, :w],
                            mybir.ActivationFunctionType.Exp, scale=SCALE)
                        # ctx^T accumulation, chunks aligned to psx banks
                        lo = g0 - t0
                        off = 0
                        while off < w:
                            lc = lo + off
                            nw = min(512 - lc % 512, w - off)
                            m_last = min(m_hi - 1, (g0 + off + nw - 1)
                                         // 128)
                            nc.tensor.matmul(
                                psx[:, lc:lc + nw], vo[a][:, m, :],
                                at[:, off:off + nw],
                                start=(m == 0), stop=(m == m_last),
                                skip_group_check=True)
                            off += nw

                    # normalize: copy denominator row to a partition-0
                    # SBUF row (the approx reciprocal ignores partition
                    # offsets), fast reciprocal, GpSimd broadcast
                    rcp = np_.tile([1, NH], F32, name="rcp", tag="rcp")
                    nc.vector.tensor_copy(rcp[:], psx[64:65, :])
                    rr = np_.tile([1, NH], F32, name="rr", tag="rr")
                    nc.vector.reciprocal_approx_fast(rr[:], rcp[:])
                    rb = np_.tile([64, NH], F32, name="rb", tag="rb")
                    nc.gpsimd.partition_broadcast(rb[:], rr[:])
                    pk = l // 2
                    if l % 2 == 0:
                        nc.vector.tensor_mul(ctxT[pk][0:64, t0:t1c],
                                             psx[0:64, :], rb[:])
                    else:
                        nc.vector.tensor_mul(ctxT[pk][64:128, t0:t1c],
                                             psx[0:64, :], rb[:])

                for h in range(2):
                    for l in range(8):
                        attn_head(h, l)

            # ---------------- phase C: output projection ----------------
            with tc.tile_pool(name="ps_o", bufs=2, space="PSUM") as ps_o, \
                 tc.tile_pool(name="outp", bufs=2) as op_:
                for tb_i in range(NT):
                    pso = ps_o.tile([128, N], F32, name="pso", tag="pso")
                    for k4 in range(4):
                        lhsT = ctxT[k4][:, tb_i * 128:(tb_i + 1) * 128]
                        for nk in range(4):
                            nc.tensor.matmul(
                                pso[:, nk * 512:(nk + 1) * 512], lhsT,
                                wo_sb[:, k4, nk * 512:(nk + 1) * 512],
                                start=(k4 == 0), stop=(k4 == 3))
                    ost = op_.tile([128, N], BF16, name="ost", tag="ost")
                    if tb_i % 2 == 0:
                        nc.scalar.copy(ost[:], pso[:])
                    else:
                        nc.vector.tensor_copy(ost[:], pso[:])
                    nc.sync.dma_start(
                        out_d[tb_i * 128:(tb_i + 1) * 128, :], ost[:])

    nc.compile()
    return nc


_NC_CACHE = {}


def _get_nc():
    if "nc" not in _NC_CACHE:
        _NC_CACHE["nc"] = _build_program()
    return _NC_CACHE["nc"]


def kernel(x, cos, sin, mask, Wq, Wk, Wv, Wo, _trace=False, _trace_kwargs=None):
    x = np.asarray(x, dtype=np.float32)
    cos = np.asarray(cos, dtype=np.float32)
    sin = np.asarray(sin, dtype=np.float32)
    Wq = np.asarray(Wq, dtype=np.float32)
    Wk = np.asarray(Wk, dtype=np.float32)
    Wv = np.asarray(Wv, dtype=np.float32)
    Wo = np.asarray(Wo, dtype=np.float32)

    nc = _get_nc()
    in_maps = []
    for c in range(8):
        bidx = c // 4
        p = c % 4
        in_maps.append({
            "x": np.ascontiguousarray(x[bidx]),
            "cos": cos,
            "sin": sin,
            "wq": np.ascontiguousarray(Wq[:, p * 512:(p + 1) * 512]),
            "wk": np.ascontiguousarray(Wk[:, p * 128:(p + 1) * 128]),
            "wv": np.ascontiguousarray(Wv[:, p * 128:(p + 1) * 128]),
            "wo": np.ascontiguousarray(Wo[p * 512:(p + 1) * 512, :]),
        })

    kwargs = {}
    if _trace:
        kwargs["trace"] = True
        kwargs.update(_trace_kwargs or {})
    res = run_bass_kernel_spmd(nc, in_maps, core_ids=list(range(8)), **kwargs)
    parts = [np.asarray(r["out"], dtype=np.float32) for r in res.results]
    out = np.stack([
        parts[0] + parts[1] + parts[2] + parts[3],
        parts[4] + parts[5] + parts[6] + parts[7],
    ]).astype(np.float32)
    if _trace:
        kernel._last_result = res
    return out


# revision 18
# speedup vs baseline: 1.0659x; 1.0659x over previous
"""GQA attention kernel for 8 Trainium2 NeuronCores.

Problem: B=2, N=2048, D=2048, H=32 heads, G=8 KV groups, head_dim=64, RoPE,
causal mask, fused QKV/output projections.

Sharding: one (batch, group-pair) unit per core — core c handles batch c//4
and KV groups {2*(c%4), 2*(c%4)+1} (8 query heads). Each core computes a
partial output projection (its heads' rows of Wo); the host sums the 4
partials per batch.

Per-core pipeline (all matmuls in bf16, fp32 accumulate):
  x --cast DMA--> xbf (DRAM, bf16) --xbar-transpose DMA--> xT [din, tok]
  QKV projections (lhsT = xT blocks), RoPE in natural layout on DVE,
  PE-transpose q/k to q^T/k^T [d, tok], then attention per token-half
  (1024 tokens) per head, key-block-major:
    scores^T[m] = k^T_m.T @ q^T  (PSUM) -> exp on ACT -> attn^T (bf16 SBUF)
    causal: skip key blocks above the diagonal; triangular mask on diag block
    ctx^T += [v_m | 1].T @ attn^T_m  -> row 64 = softmax denominators
  normalize: reciprocal_approx_fast on the denominator row, gpsimd
  partition_broadcast across 64 partitions, one DVE multiply into ctx^T
  (psx double-buffered so the tail overlaps the next head's matmuls),
  out = ctx^T.T @ Wo in bf16 partials summed on host.
"""

import numpy as np

import concourse.bass as bass
import concourse.bacc as bacc
import concourse.mybir as mybir
import concourse.tile as tile
from concourse.bass_utils import run_bass_kernel_spmd
from concourse.masks import make_identity, make_upper_triangular

F32 = mybir.dt.float32
BF16 = mybir.dt.bfloat16

N = 2048          # sequence length
D = 2048          # model dim
HD = 64           # head dim
QF = 512          # q features per core (8 heads)
KF = 128          # k/v features per core (2 groups)
NT = N // 128     # token blocks
KC = D // 128     # contraction chunks
NH = N // 2       # tokens per attention half
SCALE = 1.0 / 8.0  # 1/sqrt(HD)


def _build_program():
    nc = bacc.Bacc("TRN2", debug=False, target_bir_lowering=False)

    x_d = nc.dram_tensor("x", [N, D], F32, kind="ExternalInput")
    cos_d = nc.dram_tensor("cos", [N, HD], F32, kind="ExternalInput")
    sin_d = nc.dram_tensor("sin", [N, HD], F32, kind="ExternalInput")
    wq_d = nc.dram_tensor("wq", [D, QF], F32, kind="ExternalInput")
    wk_d = nc.dram_tensor("wk", [D, KF], F32, kind="ExternalInput")
    wv_d = nc.dram_tensor("wv", [D, KF], F32, kind="ExternalInput")
    wo_d = nc.dram_tensor("wo", [QF, D], F32, kind="ExternalInput")
    out_d = nc.dram_tensor("out", [N, D], BF16, kind="ExternalOutput")

    with tile.TileContext(nc) as tc:
        with tc.tile_pool(name="persist", bufs=1) as pp:
            # persistent SBUF: q^T/k^T, [v|1], ctx^T, wo, constants
            qT = [pp.tile([128, N], BF16, name=f"qT{t}") for t in range(4)]
            kT = pp.tile([128, N], BF16, name="kT")
            vo = [pp.tile([128, NT, HD + 1], BF16, name=f"vo{g}") for g in range(2)]
            ctxT = [pp.tile([128, N], BF16, name=f"ctxT{k}") for k in range(4)]
            wo_sb = pp.tile([128, 4, N], BF16, name="wo_sb")
            ident = pp.tile([128, 128], BF16, name="ident")
            maskt = pp.tile([128, 128], BF16, name="maskt")

            make_identity(nc, ident)
            make_upper_triangular(nc, maskt, val=1.0, diag=True)
            for g in range(2):
                nc.vector.memset(vo[g][:, :, HD:HD + 1], 1.0)

            nc.gpsimd.dma_start(
                wo_sb[:], wo_d[:].rearrange("(ko ki) n -> ki ko n", ki=128))

            # ---------------- phase A: x^T + projections + rope ----------
            with tc.tile_pool(name="phaseA", bufs=1) as pa, \
                 tc.tile_pool(name="dram", bufs=1, space="DRAM") as dp, \
                 tc.tile_pool(name="ps_q", bufs=2, space="PSUM") as ps_q, \
                 tc.tile_pool(name="ps_kv", bufs=2, space="PSUM") as ps_kv, \
                 tc.tile_pool(name="ps_tr", bufs=2, space="PSUM") as ps_tr, \
                 tc.tile_pool(name="ropetmp", bufs=6) as rtp:

                xbf = dp.tile([N, D], BF16, name="xbf")
                xT = [pa.tile([128, N], BF16, name=f"xT{kc}")
                      for kc in range(KC)]
                wq_sb = pa.tile([128, KC, QF], BF16, name="wq_sb")
                wkv_sb = pa.tile([128, KC, 2 * KF], BF16, name="wkv_sb")
                cos_sb = pa.tile([128, NT, HD], F32, name="cos_sb")
                sin_sb = pa.tile([128, NT, HD], F32, name="sin_sb")
                q_rope = pa.tile([128, NT, QF], BF16, name="q_rope")
                k_rope = pa.tile([128, NT, KF], BF16, name="k_rope")

                nc.gpsimd.dma_start(
                    wq_sb[:], wq_d[:].rearrange("(ko ki) n -> ki ko n", ki=128))
                nc.gpsimd.dma_start(
                    wkv_sb[:, :, 0:KF],
                    wk_d[:].rearrange("(ko ki) n -> ki ko n", ki=128))
                nc.gpsimd.dma_start(
                    wkv_sb[:, :, KF:2 * KF],
                    wv_d[:].rearrange("(ko ki) n -> ki ko n", ki=128))
                nc.sync.dma_start(
                    cos_sb[:], cos_d[:].rearrange("(t p) d -> p t d", p=128))
                nc.sync.dma_start(
                    sin_sb[:], sin_d[:].rearrange("(t p) d -> p t d", p=128))

                # cast x to bf16 in DRAM in 4 column groups so the
                # xbar-transposes (which need all rows of their column
                # block) can start after 1/4 of the cast
                for cg in range(4):
                    nc.gpsimd.dma_start(xbf[:, cg * 512:(cg + 1) * 512],
                                        x_d[:, cg * 512:(cg + 1) * 512])
                for kc in range(KC):
                    nc.sync.dma_start_transpose(
                        xT[kc][:], xbf[:, kc * 128:(kc + 1) * 128])

                def rope(ps, cos_b, sin_b, out_v, ab_shape):
                    """ps 4D view [128, *ab, 2, 32]; cos_b/sin_b broadcast
                    [128, *ab, 32]; out_v same 4D view layout as ps."""
                    q1 = ps[..., 0, :]
                    q2 = ps[..., 1, :]
                    c1 = cos_b[0]
                    c2 = cos_b[1]
                    s1 = sin_b[0]
                    s2 = sin_b[1]
                    ta = rtp.tile([128] + ab_shape + [32], F32, name="rt", tag="rt")
                    tb = rtp.tile([128] + ab_shape + [32], F32, name="rt", tag="rt")
                    nc.vector.tensor_mul(ta[:], q1, c1)
                    nc.vector.tensor_mul(tb[:], q2, s1)
                    nc.vector.tensor_sub(out_v[..., 0, :], ta[:], tb[:])
                    tc_ = rtp.tile([128] + ab_shape + [32], F32, name="rt", tag="rt")
                    td = rtp.tile([128] + ab_shape + [32], F32, name="rt", tag="rt")
                    nc.vector.tensor_mul(tc_[:], q2, c2)
                    nc.vector.tensor_mul(td[:], q1, s2)
                    nc.vector.tensor_add(out_v[..., 1, :], tc_[:], td[:])

                for tb_i in range(NT):
                    psq = ps_q.tile([128, QF], F32, name="psq", tag="psq")
                    pskv = ps_kv.tile([128, 2 * KF], F32, name="pskv", tag="pskv")
                    for kc in range(KC):
                        lhsT = xT[kc][:, tb_i * 128:(tb_i + 1) * 128]
                        st = kc == 0
                        sp = kc == KC - 1
                        nc.tensor.matmul(psq[:], lhsT, wq_sb[:, kc, :],
                                         start=st, stop=sp)
                        nc.tensor.matmul(pskv[:], lhsT, wkv_sb[:, kc, :],
                                         start=st, stop=sp)

                    # --- RoPE Q: psq cols = a*256 + b*64 + h*32 + j
                    #     out cols = b*128 + a*64 + h*32 + j (head pairs
                    #     (t, t+4) adjacent for the transpose step)
                    psq_v = psq[:].rearrange("p (a b h j) -> p a b h j",
                                             a=2, b=4, h=2)
                    out_v = q_rope[:, tb_i, :].rearrange(
                        "p (b a h j) -> p a b h j", b=4, a=2, h=2)
                    cs = cos_sb[:, tb_i, :]
                    sn = sin_sb[:, tb_i, :]

                    def bcq(apv):
                        return apv.unsqueeze(1).unsqueeze(1).broadcast_to(
                            (128, 2, 4, 32))

                    rope(psq_v,
                         (bcq(cs[:, 0:32]), bcq(cs[:, 32:64])),
                         (bcq(sn[:, 0:32]), bcq(sn[:, 32:64])),
                         out_v, [2, 4])

                    # --- RoPE K: cols = g*64 + h*32 + j (no interleave)
                    psk_v = pskv[:, 0:KF].rearrange("p (g h j) -> p g h j",
                                                    g=2, h=2)
                    outk_v = k_rope[:, tb_i, :].rearrange(
                        "p (g h j) -> p g h j", g=2, h=2)

                    def bck(apv):
                        return apv.unsqueeze(1).broadcast_to((128, 2, 32))

                    rope(psk_v,
                         (bck(cs[:, 0:32]), bck(cs[:, 32:64])),
                         (bck(sn[:, 0:32]), bck(sn[:, 32:64])),
                         outk_v, [2])

                    # --- V -> bf16 SBUF with ones column
                    for g in range(2):
                        nc.scalar.copy(vo[g][:, tb_i, 0:HD],
                                       pskv[:, KF + g * 64:KF + (g + 1) * 64])

                    # --- PE transposes: q_rope/k_rope -> qT/kT
                    for t in range(4):
                        ptr = ps_tr.tile([128, 128], BF16, name="ptr", tag="ptr")
                        nc.tensor.transpose(
                            ptr[:], q_rope[:, tb_i, t * 128:(t + 1) * 128],
                            ident[:])
                        nc.vector.tensor_copy(
                            qT[t][:, tb_i * 128:(tb_i + 1) * 128], ptr[:])
                    ptrk = ps_tr.tile([128, 128], BF16, name="ptr", tag="ptr")
                    nc.tensor.transpose(ptrk[:], k_rope[:, tb_i, :], ident[:])
                    nc.scalar.copy(kT[:, tb_i * 128:(tb_i + 1) * 128], ptrk[:])

            # ---------------- phase B: attention ------------------------
            # token-half-major so the normalization tail of head l
            # overlaps the matmuls of head l+1 (psx double-buffered)
            with tc.tile_pool(name="ps_sc", bufs=2, space="PSUM") as ps_sc, \
                 tc.tile_pool(name="ps_cx", bufs=2, space="PSUM") as ps_cx, \
                 tc.tile_pool(name="attnp", bufs=3) as ap_, \
                 tc.tile_pool(name="normp", bufs=2) as np_:

                def attn_head(h, l):
                    t0 = h * NH
                    t1c = (h + 1) * NH
                    m_hi = (h + 1) * (NH // 128)   # key blocks 0..m_hi-1
                    a, b = l // 4, l % 4
                    r0 = 64 * a
                    psx = ps_cx.tile([HD + 1, NH], F32, name="psx",
                                     tag="psx")
                    for m in range(m_hi):
                        g0 = max(m * 128, t0)
                        w = t1c - g0
                        lhs_k = kT[r0:r0 + 64, m * 128:(m + 1) * 128]
                        psc = ps_sc.tile([128, NH], F32, name="psc",
                                         tag="psc")
                        off = 0
                        while off < w:
                            nw = min(512, w - off)
                            nc.tensor.matmul(
                                psc[:, off:off + nw], lhs_k,
                                qT[b][r0:r0 + 64,
                                      g0 + off:g0 + off + nw],
                                start=True, stop=True)
                            off += nw
                        at = ap_.tile([128, NH], BF16, name="at",
                                      tag="at")
                        nc.scalar.activation(
                            at[:, :w], psc[:, :w],
                            mybir.ActivationFunctionType.Exp, scale=SCALE)
                        if g0 == m * 128:
                            nc.vector.tensor_mul(
                                at[:, 0:128], at[:, 0:128], maskt[:])
                        # ctx^T accumulation, chunks aligned to psx banks
                        lo = g0 - t0
                        off = 0
                        while off < w:
                            lc = lo + off
                            nw = min(512 - lc % 512, w - off)
                            m_last = min(m_hi - 1, (g0 + off + nw - 1)
                                         // 128)
                            nc.tensor.matmul(
                                psx[:, lc:lc + nw], vo[a][:, m, :],
                                at[:, off:off + nw],
                                start=(m == 0), stop=(m == m_last),
                                skip_group_check=True)
                            off += nw

                    # normalize: copy denominator row to a partition-0
                    # SBUF row (the approx reciprocal ignores partition
                    # offsets), fast reciprocal, GpSimd broadcast
                    rcp = np_.tile([1, NH], F32, name="rcp", tag="rcp")
                    nc.vector.tensor_copy(rcp[:], psx[64:65, :])
                    rr = np_.tile([1, NH], F32, name="rr", tag="rr")
                    nc.vector.reciprocal_approx_fast(rr[:], rcp[:])
                    rb = np_.tile([64, NH], F32, name="rb", tag="rb")
                    nc.gpsimd.partition_broadcast(rb[:], rr[:])
                    pk = l // 2
                    if l % 2 == 0:
                        nc.vector.tensor_mul(ctxT[pk][0:64, t0:t1c],
                                             psx[0:64, :], rb[:])
                    else:
                        nc.vector.tensor_mul(ctxT[pk][64:128, t0:t1c],
                                             psx[0:64, :], rb[:])

                for h in range(2):
                    for l in range(8):
                        attn_head(h, l)

            # ---------------- phase C: output projection ----------------
            with tc.tile_pool(name="ps_o", bufs=2, space="PSUM") as ps_o, \
                 tc.tile_pool(name="outp", bufs=2) as op_:
                for tb_i in range(NT):
                    pso = ps_o.tile([128, N], F32, name="pso", tag="pso")
                    for k4 in range(4):
                        lhsT = ctxT[k4][:, tb_i * 128:(tb_i + 1) * 128]
                        for nk in range(4):
                            nc.tensor.matmul(
                                pso[:, nk * 512:(nk + 1) * 512], lhsT,
                                wo_sb[:, k4, nk * 512:(nk + 1) * 512],
                                start=(k4 == 0), stop=(k4 == 3))
                    ost = op_.tile([128, N], BF16, name="ost", tag="ost")
                    if tb_i % 2 == 0:
                        nc.scalar.copy(ost[:], pso[:])
                    else:
                        nc.vector.tensor_copy(ost[:], pso[:])
                    nc.sync.dma_start(
                        out_d[tb_i * 128:(tb_i + 1) * 128, :], ost[:])

    nc.compile()
    return nc


_NC_CACHE = {}


def _get_nc():
    if "nc" not in _NC_CACHE:
        _NC_CACHE["nc"] = _build_program()
    return _NC_CACHE["nc"]


def kernel(x, cos, sin, mask, Wq, Wk, Wv, Wo, _trace=False, _trace_kwargs=None):
    x = np.asarray(x, dtype=np.float32)
    cos = np.asarray(cos, dtype=np.float32)
    sin = np.asarray(sin, dtype=np.float32)
    Wq = np.asarray(Wq, dtype=np.float32)
    Wk = np.asarray(Wk, dtype=np.float32)
    Wv = np.asarray(Wv, dtype=np.float32)
    Wo = np.asarray(Wo, dtype=np.float32)

    nc = _get_nc()
    in_maps = []
    for c in range(8):
        bidx = c // 4
        p = c % 4
        in_maps.append({
            "x": np.ascontiguousarray(x[bidx]),
            "cos": cos,
            "sin": sin,
            "wq": np.ascontiguousarray(Wq[:, p * 512:(p + 1) * 512]),
            "wk": np.ascontiguousarray(Wk[:, p * 128:(p + 1) * 128]),
            "wv": np.ascontiguousarray(Wv[:, p * 128:(p + 1) * 128]),
            "wo": np.ascontiguousarray(Wo[p * 512:(p + 1) * 512, :]),
        })

    kwargs = {}
    if _trace:
        kwargs["trace"] = True
        kwargs.update(_trace_kwargs or {})
    res = run_bass_kernel_spmd(nc, in_maps, core_ids=list(range(8)), **kwargs)
    parts = [np.asarray(r["out"], dtype=np.float32) for r in res.results]
    out = np.stack([
        parts[0] + parts[1] + parts[2] + parts[3],
        parts[4] + parts[5] + parts[6] + parts[7],
    ]).astype(np.float32)
    if _trace:
        kernel._last_result = res
    return out
